# revision 1
# baseline (speedup 1.0000x reference)
"""HSTU-style attention block (RoPE + multi-scale temporal agg + SDPA + LN + out-proj)
for Trainium2, data-parallel over batch across 8 NeuronCores.

Per-core layout strategy (batch element per core):
  - host pre-transposes X so projections run with activations as lhsT
  - Q/K/V projected into natural [s, h'] layout; RoPE applied with strided DVE ops
  - temporal aggregation applied as a matmul against a host-built [S, S] matrix T
    (derived from softmax(temporal_weights)); Q/K produced transposed, V natural
    with an extra ones column so softmax denominators ride the PV matmul
  - attention computes scores^T per head (contraction zero-padded to K=128 to
    keep the PE clock warm), Exp on the scalar engine, PV accumulation over key
    chunks; softmax normalization + LayerNorm statistics fold into the head loop
  - LayerNorm runs across partitions (PE ones-matmul sums), out-projection natural
All matmuls run in float32r (TF32-like precision, ~2x bf16 cycle cost).
"""

import numpy as np
import concourse.mybir as mybir
import concourse.tile as tile
from concourse import bacc
from concourse.bass_utils import run_bass_kernel_spmd

B, S, H, NH = 8, 1024, 1024, 16
HD = H // NH  # 64
P = 128
SO = S // P  # 8
HO = H // P  # 8
N_SCALES = 4
LN_EPS = 1e-5
F32 = mybir.dt.float32
F32R = mybir.dt.float32r

N_CORES = 8


# ---------------------------------------------------------------- host helpers
def _softmax_np(x):
    x = np.asarray(x, np.float64)
    e = np.exp(x - x.max())
    return e / e.sum()


def _temporal_matrix(temporal_weights):
    """[S, S] matrix T with (T @ x) == temporal_agg(x) along the sequence axis."""
    w = _softmax_np(temporal_weights)
    T = np.eye(S, dtype=np.float64) * w[0]
    for scale in range(1, N_SCALES):
        p = max(1, S // (2 ** scale))
        k = S // p
        pool = np.zeros((p, S), dtype=np.float64)
        for j in range(p):
            pool[j, j * k:(j + 1) * k] = 1.0 / k
        coord = (np.arange(S, dtype=np.float64) + 0.5) * (p / S) - 0.5
        coord = np.clip(coord, 0.0, None)
        i0 = np.minimum(np.floor(coord).astype(np.int64), p - 1)
        i1 = np.minimum(i0 + 1, p - 1)
        lam = (coord - i0).astype(np.float32).astype(np.float64)
        interp = np.zeros((S, p), dtype=np.float64)
        interp[np.arange(S), i0] += 1.0 - lam
        interp[np.arange(S), i1] += lam
        T += w[scale] * (interp @ pool)
    return T.astype(np.float32)


def _rope_tables():
    inv_freq = 1.0 / (10000.0 ** (np.arange(0, HD, 2, dtype=np.float64) / HD))
    freqs = np.arange(S, dtype=np.float64)[:, None] * inv_freq[None, :]
    cos = np.repeat(np.cos(freqs), 2, axis=-1).astype(np.float32)  # [S, HD]
    sin = np.repeat(np.sin(freqs), 2, axis=-1).astype(np.float32)
    return cos, sin


def _nat(x):
    """[S, D] -> [P, S//P, D] with x[so*P+p, d] = out[p, so, d]."""
    return np.ascontiguousarray(x.reshape(SO, P, x.shape[-1]).transpose(1, 0, 2))


def _xt_chunks(x):
    """[S, H] -> [P, SO, HO*P] with out[p, so, ho*P + i] = x[so*P + i, ho*P + p]."""
    return np.ascontiguousarray(
        x.reshape(SO, P, HO, P).transpose(3, 0, 2, 1).reshape(P, SO, H))


# ---------------------------------------------------------------- bass program
def _build_program():
    nc = bacc.Bacc("TRN2", target_bir_lowering=False, debug=False)

    d_xt = {a: nc.dram_tensor(f"xt_{a}", [P, SO, H], F32R, kind="ExternalInput")
            for a in ("v", "q", "k")}
    d_w = {a: nc.dram_tensor(f"w_{a}", [P, HO, H], F32R, kind="ExternalInput")
           for a in ("v", "q", "k", "o")}
    d_b = {a: nc.dram_tensor(f"b_{a}", [1, H], F32, kind="ExternalInput")
           for a in ("v", "q", "k", "o")}
    d_tt = nc.dram_tensor("tt", [P, SO, S], F32R, kind="ExternalInput")
    d_cos = nc.dram_tensor("cos_t", [P, SO, HD], F32, kind="ExternalInput")
    d_sin = nc.dram_tensor("sin_t", [P, SO, HD], F32, kind="ExternalInput")
    d_gam = nc.dram_tensor("ln_g", [P, HO], F32, kind="ExternalInput")
    d_bet = nc.dram_tensor("ln_b", [P, HO], F32, kind="ExternalInput")
    d_y = nc.dram_tensor("y", [P, SO, H], F32, kind="ExternalOutput")
    d_zp = nc.dram_tensor("zpad", [HD, S], F32R, kind="ExternalInput")
    # per-chunk scratch so a head's reload only waits on its own spill DMA
    d_qs = [nc.dram_tensor(f"q_scr{hc}", [P, S], F32R) for hc in range(HO)]
    d_ks = [nc.dram_tensor(f"k_scr{hc}", [P, S], F32R) for hc in range(HO)]

    with tile.TileContext(nc) as tc:
        with (
            tc.tile_pool(name="const", bufs=1) as cpool,
            tc.tile_pool(name="big", bufs=4) as big,
            tc.tile_pool(name="s4", bufs=10) as s4,
            tc.tile_pool(name="s2", bufs=6) as s2,
            tc.tile_pool(name="mm_ps", bufs=4, space="PSUM") as mmps,
            tc.tile_pool(name="sc_ps", bufs=2, space="PSUM") as scps,
            tc.tile_pool(name="pv_ps", bufs=2, space="PSUM") as pvps,
        ):
            cos_t = cpool.tile([P, SO, HD], F32, name="cos_t")
            sin_t = cpool.tile([P, SO, HD], F32, name="sin_t")
            nc.sync.dma_start(cos_t[:], d_cos.ap())
            nc.sync.dma_start(sin_t[:], d_sin.ap())
            gam_t = cpool.tile([P, HO], F32, name="gam_t")
            bet_t = cpool.tile([P, HO], F32, name="bet_t")
            nc.sync.dma_start(gam_t[:], d_gam.ap())
            nc.sync.dma_start(bet_t[:], d_bet.ap())
            ones = cpool.tile([P, 1], F32, name="ones")
            nc.vector.memset(ones[:], 1.0)
            eps_t = cpool.tile([P, 1], F32, name="eps_t")
            nc.vector.memset(eps_t[:], LN_EPS)

            def _rope_chunk(a_nat, so):
                ch = a_nat[:, so, :]
                ch3 = ch.rearrange("p (nh d) -> p nh d", d=HD)
                ch4 = ch.rearrange("p (nh hf dd) -> p nh hf dd", hf=2, dd=HD // 2)
                rot = s4.tile([P, H], F32, tag="s4")
                rot4 = rot[:].rearrange("p (nh hf dd) -> p nh hf dd",
                                        hf=2, dd=HD // 2)
                rot3 = rot[:].rearrange("p (nh d) -> p nh d", d=HD)
                nc.vector.tensor_scalar_mul(rot4[:, :, 0, :], ch4[:, :, 1, :], -1.0)
                nc.vector.tensor_copy(rot4[:, :, 1, :], ch4[:, :, 0, :])
                cb = cos_t[:, so, :][:, None, :].to_broadcast((P, NH, HD))
                sb = sin_t[:, so, :][:, None, :].to_broadcast((P, NH, HD))
                nc.vector.tensor_tensor(ch3[:], ch3[:], cb, mybir.AluOpType.mult)
                nc.vector.tensor_tensor(rot3[:], rot3[:], sb, mybir.AluOpType.mult)
                nc.vector.tensor_tensor(ch[:], ch[:], rot[:], mybir.AluOpType.add)

            def project(a, do_rope=False):
                """A_nat [P, SO, H] (f32r) = X @ W_a + b_a, optional fused RoPE.

                RoPE is interleaved per s-chunk so DVE work tracks PE work and
                the tensor engine never idles long enough to drop its clock."""
                w_t = big.tile([P, HO, H], F32R, tag="big")
                nc.sync.dma_start(w_t[:], d_w[a].ap())
                brow = s4.tile([1, H], F32, tag="s4")
                nc.sync.dma_start(brow[:], d_b[a].ap())
                bb = s4.tile([P, H], F32, tag="s4")
                nc.gpsimd.partition_broadcast(bb[:], brow[:])
                a_nat = big.tile([P, SO, H], F32R, tag="big")
                for so in range(SO):
                    xt_c = s4.tile([P, HO, P], F32R, tag="s4")
                    nc.sync.dma_start(xt_c[:], d_xt[a].ap()[:, so, :])
                    for nh in range(2):
                        ps = mmps.tile([P, 512], F32, tag="mm")
                        for ko in range(HO):
                            nc.tensor.matmul(
                                ps[:], xt_c[:, ko, :],
                                w_t[:, ko, nh * 512:(nh + 1) * 512],
                                start=(ko == 0), stop=(ko == HO - 1))
                        nc.vector.tensor_tensor(
                            a_nat[:, so, nh * 512:(nh + 1) * 512], ps[:],
                            bb[:, nh * 512:(nh + 1) * 512], mybir.AluOpType.add)
                    if do_rope:
                        _rope_chunk(a_nat, so)
                return a_nat

            BAND = 12  # T[s', s] == 0 for |s' - s| > 11 (structural)

            def _band_sos(o0, o1):
                """so chunks whose s-range intersects [o0-BAND, o1+BAND)."""
                return [so for so in range(SO)
                        if so * P + P > o0 - BAND and so * P < o1 + BAND]

            def t_agg_spill(a_nat, tt, d_scr):
                """(T @ A).T evicted through SBUF chunks into DRAM scratch.
                Evictions ride the scalar engine -- idle during phase 1."""
                for hc in range(HO):
                    for sh in range(2):
                        sos = _band_sos(sh * 512, (sh + 1) * 512)
                        ps = mmps.tile([P, 512], F32, tag="mm")
                        for so in sos:
                            nc.tensor.matmul(
                                ps[:], a_nat[:, so, hc * P:(hc + 1) * P],
                                tt[:, so, sh * 512:(sh + 1) * 512],
                                start=(so == sos[0]), stop=(so == sos[-1]))
                        ev = s2.tile([P, 512], F32R, tag="s2")
                        nc.scalar.copy(ev[:], ps[:])
                        nc.sync.dma_start(
                            d_scr[hc].ap()[:, sh * 512:(sh + 1) * 512], ev[:])

            def t_agg_v(v_nat, tt):
                """V_ext [P, SO, NH, HD+1] (f32r) = T @ V with ones column."""
                v_ext = big.tile([P, SO, NH, HD + 1], F32R, tag="big")
                nc.vector.tensor_copy(
                    v_ext[:, :, :, HD:HD + 1],
                    ones[:, None, None, :].to_broadcast((P, SO, NH, 1)))
                for sc in range(SO):
                    sos = _band_sos(sc * P, (sc + 1) * P)
                    for dh in range(2):
                        ps = mmps.tile([P, 512], F32, tag="mm")
                        for so in sos:
                            nc.tensor.matmul(
                                ps[:], tt[:, so, sc * P:(sc + 1) * P],
                                v_nat[:, so, dh * 512:(dh + 1) * 512],
                                start=(so == sos[0]), stop=(so == sos[-1]))
                        pvw = ps[:].rearrange("p (nh d) -> p nh d", d=HD)
                        nc.scalar.copy(
                            v_ext[:, sc, dh * 8:(dh + 1) * 8, 0:HD], pvw)
                return v_ext

            # ---- phase 1: V, Q, K  (projection + RoPE + temporal aggregation)
            v_nat = project("v")
            tt = big.tile([P, SO, S], F32R, tag="big")
            nc.sync.dma_start(tt[:], d_tt.ap())
            v_ext = t_agg_v(v_nat, tt)

            q_nat = project("q", do_rope=True)
            t_agg_spill(q_nat, tt, d_qs)

            k_nat = project("k", do_rope=True)
            t_agg_spill(k_nat, tt, d_ks)

            # ---- phase 2: attention (normalization + LN stats fold into the
            # head loop so the tail barrier shrinks to the final LN apply)
            attn_T = big.tile([P, HO, S], F32, tag="big")
            acc = s4.tile([P, S], F32R, tag="s4")
            acc2 = s4.tile([P, S], F32R, tag="s4")
            rb_c = None
            for h in range(NH):
                hc, off = h // 2, (h % 2) * HD
                # zero-pad the contraction dim to K=128: half-array (K=64)
                # matmuls never trip the PE activity monitor, pinning the
                # clock at 1.2 GHz. Rows 64:128 come from a DRAM zeros pad.
                kh = s4.tile([P, S], F32R, tag="s4")
                nc.sync.dma_start(kh[0:HD, :], d_ks[hc].ap()[off:off + HD, :])
                nc.sync.dma_start(kh[HD:P, :], d_zp.ap())
                qh = s4.tile([P, S], F32R, tag="s4")
                nc.sync.dma_start(qh[0:HD, :], d_qs[hc].ap()[off:off + HD, :])
                nc.sync.dma_start(qh[HD:P, :], d_zp.ap())
                if off == 0:
                    rb_c = s4.tile([P, S], F32, tag="s4")
                for q2 in range(2):
                    pv = pvps.tile([P, 512], F32, tag="pv")
                    # software-pipelined: scores run one kc ahead of PV
                    ets = []
                    for kc in range(SO + 1):
                        if kc < SO:
                            sp = scps.tile([P, 512], F32, tag="sc")
                            nc.tensor.matmul(
                                sp[:], kh[0:P, kc * P:(kc + 1) * P],
                                qh[0:P, q2 * 512:(q2 + 1) * 512],
                                start=True, stop=True, skip_group_check=True)
                            e_t = s2.tile([P, 512], F32R, tag="s2")
                            nc.scalar.activation(
                                e_t[:], sp[:],
                                mybir.ActivationFunctionType.Exp, scale=0.125)
                            ets.append(e_t)
                        if kc > 0:
                            j = kc - 1
                            nc.tensor.matmul(
                                pv[0:HD + 1, :], v_ext[:, j, h, :], ets[j][:],
                                start=(j == 0), stop=(j == SO - 1),
                                skip_group_check=True)
                    # evict raw out + sums; broadcast sums (no PE dependency).
                    # partition_broadcast only writes reliably at partition 0,
                    # so odd heads bounce through a temp + DVE copy.
                    qs = slice(q2 * 512, (q2 + 1) * 512)
                    nc.vector.tensor_copy(attn_T[off:off + HD, hc, qs], pv[0:HD, :])
                    srow = s2.tile([1, 512], F32, tag="s2")
                    nc.vector.tensor_copy(srow[:], pv[HD:HD + 1, :])
                    if off == 0:
                        nc.gpsimd.partition_broadcast(rb_c[0:HD, qs], srow[:])
                    else:
                        tmp = s2.tile([HD, 512], F32, tag="s2")
                        nc.gpsimd.partition_broadcast(tmp[:], srow[:])
                        nc.vector.tensor_copy(rb_c[off:off + HD, qs], tmp[:])
                if off == HD:
                    # chunk hc complete: normalize + accumulate LN stats
                    rcp_c = s4.tile([P, S], F32, tag="s4")
                    nc.vector.reciprocal_approx_fast(rcp_c[:], rb_c[:])
                    nc.vector.tensor_tensor(attn_T[:, hc, :], attn_T[:, hc, :],
                                            rcp_c[:], mybir.AluOpType.mult)
                    if hc == 0:
                        nc.vector.tensor_copy(acc[:], attn_T[:, 0, :])
                        nc.vector.tensor_tensor(acc2[:], attn_T[:, 0, :],
                                                attn_T[:, 0, :],
                                                mybir.AluOpType.mult)
                    else:
                        nc.vector.tensor_tensor(acc[:], acc[:], attn_T[:, hc, :],
                                                mybir.AluOpType.add)
                        sqc = s4.tile([P, S], F32, tag="s4")
                        nc.vector.tensor_tensor(sqc[:], attn_T[:, hc, :],
                                                attn_T[:, hc, :],
                                                mybir.AluOpType.mult)
                        nc.vector.tensor_tensor(acc2[:], acc2[:], sqc[:],
                                                mybir.AluOpType.add)

            # prefetch out-projection weights so the DMA overlaps LayerNorm
            wo_t = big.tile([P, HO, H], F32R, tag="big")
            nc.sync.dma_start(wo_t[:], d_w["o"].ap())
            brow_o = s4.tile([1, H], F32, tag="s4")
            nc.sync.dma_start(brow_o[:], d_b["o"].ap())
            bo_b = s4.tile([P, H], F32, tag="s4")
            nc.gpsimd.partition_broadcast(bo_b[:], brow_o[:])

            # ---- phase 3: LayerNorm over h (partition axis across HO chunks)
            # partition sums via a PE ones-matmul (gpsimd allreduce is slow)
            ones_r = cpool.tile([P, 1], F32R, name="ones_r")
            nc.vector.tensor_copy(ones_r[:], ones[:])
            mu_b = s4.tile([P, S], F32, tag="s4")
            ms_b = s4.tile([P, S], F32, tag="s4")
            for src, dst in ((acc, mu_b), (acc2, ms_b)):
                for half in range(2):
                    pss = mmps.tile([P, 512], F32, tag="mm")
                    nc.tensor.matmul(pss[0:1, :], ones_r[:],
                                     src[:, half * 512:(half + 1) * 512],
                                     start=True, stop=True,
                                     skip_group_check=True)
                    srw = s2.tile([1, 512], F32, tag="s2")
                    nc.vector.tensor_copy(srw[:], pss[0:1, :])
                    nc.gpsimd.partition_broadcast(
                        dst[:, half * 512:(half + 1) * 512], srw[:])
            nc.vector.tensor_scalar_mul(mu_b[:], mu_b[:], 1.0 / H)
            nc.vector.tensor_scalar_mul(ms_b[:], ms_b[:], 1.0 / H)
            m2 = s4.tile([P, S], F32, tag="s4")
            nc.scalar.square(m2[:], mu_b[:])
            nc.vector.tensor_tensor(ms_b[:], ms_b[:], m2[:], mybir.AluOpType.subtract)
            nc.scalar.activation(ms_b[:], ms_b[:], mybir.ActivationFunctionType.Sqrt,
                                 bias=eps_t[:])
            rstd = s4.tile([P, S], F32, tag="s4")
            nc.vector.reciprocal_approx_fast(rstd[:], ms_b[:])

            ln_out = big.tile([P, HO, S], F32R, tag="big")
            for hc in range(HO):
                t1 = s4.tile([P, S], F32, tag="s4")
                nc.vector.tensor_tensor(t1[:], attn_T[:, hc, :], mu_b[:],
                                        mybir.AluOpType.subtract)
                nc.vector.tensor_tensor(t1[:], t1[:], rstd[:],
                                        mybir.AluOpType.mult)
                nc.vector.tensor_scalar(ln_out[:, hc, :], t1[:],
                                        gam_t[:, hc:hc + 1], bet_t[:, hc:hc + 1],
                                        mybir.AluOpType.mult, mybir.AluOpType.add)

            # ---- phase 4: output projection
            for so in range(SO):
                for nh in range(2):
                    ps = mmps.tile([P, 512], F32, tag="mm")
                    for hc in range(HO):
                        nc.tensor.matmul(
                            ps[:], ln_out[:, hc, so * P:(so + 1) * P],
                            wo_t[:, hc, nh * 512:(nh + 1) * 512],
                            start=(hc == 0), stop=(hc == HO - 1))
                    ych = s2.tile([P, 512], F32, tag="s2")
                    nc.vector.tensor_tensor(ych[:], ps[:],
                                            bo_b[:, nh * 512:(nh + 1) * 512],
                                            mybir.AluOpType.add)
                    nc.sync.dma_start(
                        d_y.ap()[:, so, nh * 512:(nh + 1) * 512], ych[:])

    nc.compile()
    return nc


_NC = None


def _get_nc():
    global _NC
    if _NC is None:
        _NC = _build_program()
    return _NC


def _host_inputs(query, key, value, Wq, bq, Wk, bk, Wv, bv, Wo, bo,
                 temporal_weights, ln_gamma, ln_beta):
    T = _temporal_matrix(temporal_weights)
    tt_host = np.ascontiguousarray(  # TT[p, so, s'] = T[s', so*P+p]
        T.T.reshape(SO, P, S).transpose(1, 0, 2))
    cos, sin = _rope_tables()
    common = {
        "w_v": _nat(np.asarray(Wv, np.float32)),
        "w_q": _nat(np.asarray(Wq, np.float32)),
        "w_k": _nat(np.asarray(Wk, np.float32)),
        "w_o": _nat(np.asarray(Wo, np.float32)),
        "b_v": np.asarray(bv, np.float32).reshape(1, H),
        "b_q": np.asarray(bq, np.float32).reshape(1, H),
        "b_k": np.asarray(bk, np.float32).reshape(1, H),
        "b_o": np.asarray(bo, np.float32).reshape(1, H),
        "tt": tt_host,
        "zpad": np.zeros((HD, S), np.float32),
        "cos_t": _nat(cos),
        "sin_t": _nat(sin),
        "ln_g": np.ascontiguousarray(
            np.asarray(ln_gamma, np.float32).reshape(HO, P).T),
        "ln_b": np.ascontiguousarray(
            np.asarray(ln_beta, np.float32).reshape(HO, P).T),
    }
    in_maps = []
    for c in range(N_CORES):
        m = dict(common)
        m["xt_q"] = _xt_chunks(np.asarray(query[c], np.float32))
        m["xt_k"] = _xt_chunks(np.asarray(key[c], np.float32))
        m["xt_v"] = _xt_chunks(np.asarray(value[c], np.float32))
        in_maps.append(m)
    return in_maps


def kernel(query, key, value, Wq, bq, Wk, bk, Wv, bv, Wo, bo,
           temporal_weights, ln_gamma, ln_beta):
    in_maps = _host_inputs(query, key, value, Wq, bq, Wk, bk, Wv, bv, Wo, bo,
                           temporal_weights, ln_gamma, ln_beta)
    nc = _get_nc()
    res = run_bass_kernel_spmd(nc, in_maps, list(range(N_CORES)))
    out = np.empty((B, S, H), np.float32)
    for c in range(N_CORES):
        y = res.results[c]["y"]  # [P, SO, H]
        out[c] = y.transpose(1, 0, 2).reshape(S, H)
    return out



# revision 16
# speedup vs baseline: 1.4124x; 1.4124x over previous
"""HSTU-style attention block (RoPE + multi-scale temporal agg + SDPA + LN + out-proj)
for Trainium2, data-parallel over batch across 8 NeuronCores.

Per-core layout strategy (batch element per core), v2:
  - all SBUF operands bf16 (same PE col-rate as f32r at N>=512, half the DMA
    and SBUF footprint); PSUM accumulation stays f32
  - projections: host pre-transposed X as lhsT, per-ko weight-chunk DMAs so the
    PE starts ~1.5us in; bias folded into the matmul as a K=1 rank-1 accumulate
  - temporal aggregation (T @ .) for Q/K runs per head-chunk INSIDE the head
    loop, evicted straight into persistent zero-padded SBUF tiles (no DRAM
    spill roundtrip); V aggregated in the prelude with a ones column so softmax
    denominators ride the PV matmul
  - attention: scores^T per head with contraction zero-padded to K=128 (keeps
    the PE activity monitor / clock boost engaged), Exp on the scalar engine in
    2-PSUM-bank pairs (halves Act instruction overhead), PV accumulation over
    key chunks; softmax normalization + LayerNorm stats fold into the head loop
  - LayerNorm folded into the out-projection: host computes Wg = gamma*Wo,
    u = colsum(Wg), c = beta@Wo + bo; device accumulates z@Wg - mu (x) u in
    PSUM (rank-1 correction), then scales by rstd (per-partition scalar) and
    adds c. The PE never waits on LayerNorm.
"""

import numpy as np
import ml_dtypes
import concourse.mybir as mybir
import concourse.tile as tile
from concourse import bacc
from concourse.bass_utils import run_bass_kernel_spmd

B, S, H, NH = 8, 1024, 1024, 16
HD = H // NH  # 64
P = 128
SO = S // P  # 8
HO = H // P  # 8
N_SCALES = 4
LN_EPS = 1e-5
F32 = mybir.dt.float32
F32R = mybir.dt.float32r
BF16 = mybir.dt.bfloat16
BF = ml_dtypes.bfloat16

N_CORES = 8
BAND = 12  # T[s', s] == 0 for |s' - s| > 11 (structural)


# ---------------------------------------------------------------- host helpers
def _softmax_np(x):
    x = np.asarray(x, np.float64)
    e = np.exp(x - x.max())
    return e / e.sum()


def _temporal_matrix(temporal_weights):
    """[S, S] matrix T with (T @ x) == temporal_agg(x) along the sequence axis."""
    w = _softmax_np(temporal_weights)
    T = np.eye(S, dtype=np.float64) * w[0]
    for scale in range(1, N_SCALES):
        p = max(1, S // (2 ** scale))
        k = S // p
        pool = np.zeros((p, S), dtype=np.float64)
        for j in range(p):
            pool[j, j * k:(j + 1) * k] = 1.0 / k
        coord = (np.arange(S, dtype=np.float64) + 0.5) * (p / S) - 0.5
        coord = np.clip(coord, 0.0, None)
        i0 = np.minimum(np.floor(coord).astype(np.int64), p - 1)
        i1 = np.minimum(i0 + 1, p - 1)
        lam = (coord - i0).astype(np.float32).astype(np.float64)
        interp = np.zeros((S, p), dtype=np.float64)
        interp[np.arange(S), i0] += 1.0 - lam
        interp[np.arange(S), i1] += lam
        T += w[scale] * (interp @ pool)
    return T.astype(np.float32)


def _rope_tables():
    inv_freq = 1.0 / (10000.0 ** (np.arange(0, HD, 2, dtype=np.float64) / HD))
    freqs = np.arange(S, dtype=np.float64)[:, None] * inv_freq[None, :]
    cos = np.repeat(np.cos(freqs), 2, axis=-1).astype(np.float32)  # [S, HD]
    sin = np.repeat(np.sin(freqs), 2, axis=-1).astype(np.float32)
    return cos, sin


def _nat(x):
    """[S, D] -> [P, S//P, D] with x[so*P+p, d] = out[p, so, d]."""
    return np.ascontiguousarray(x.reshape(SO, P, x.shape[-1]).transpose(1, 0, 2))


def _xt_chunks(x):
    """[S, H] -> [P, SO, HO*P] with out[p, so, ho*P + i] = x[so*P + i, ho*P + p]."""
    return np.ascontiguousarray(
        x.reshape(SO, P, HO, P).transpose(3, 0, 2, 1).reshape(P, SO, H))


# ---------------------------------------------------------------- bass program
def _build_program():
    nc = bacc.Bacc("TRN2", target_bir_lowering=False, debug=False)

    d_xt = {a: nc.dram_tensor(f"xt_{a}", [P, SO, H], BF16, kind="ExternalInput")
            for a in ("v", "q", "k")}
    d_w = {a: nc.dram_tensor(f"w_{a}", [P, HO, H], BF16, kind="ExternalInput")
           for a in ("v", "q", "k", "g")}
    d_b = {a: nc.dram_tensor(f"b_{a}", [1, H], BF16, kind="ExternalInput")
           for a in ("v", "q", "k")}
    d_tt = nc.dram_tensor("tt", [P, SO, S], BF16, kind="ExternalInput")
    d_cos = nc.dram_tensor("cos_t", [P, SO, HD], BF16, kind="ExternalInput")
    d_sin = nc.dram_tensor("sin_t", [P, SO, HD], BF16, kind="ExternalInput")
    d_urow = nc.dram_tensor("urow", [1, H], F32R, kind="ExternalInput")
    d_crow = nc.dram_tensor("crow", [1, H], F32, kind="ExternalInput")
    d_zp = nc.dram_tensor("zpad", [HD, S], BF16, kind="ExternalInput")
    d_y = nc.dram_tensor("y", [P, SO, H], F32, kind="ExternalOutput")

    with tile.TileContext(nc) as tc:
        with (
            tc.tile_pool(name="const", bufs=1) as cpool,
            tc.tile_pool(name="big", bufs=2) as big,
            tc.tile_pool(name="s4", bufs=4) as s4,
            tc.tile_pool(name="s2", bufs=4) as s2,
            tc.tile_pool(name="mm_ps", bufs=2, space="PSUM") as mmps,
            tc.tile_pool(name="sc_ps", bufs=2, space="PSUM") as scps,
            tc.tile_pool(name="pv_ps", bufs=2, space="PSUM") as pvps,
        ):
            # ---- constants / persistent state
            brow = {}
            for a in ("v", "q", "k"):
                brow[a] = cpool.tile([1, H], BF16, name=f"brow_{a}")
                nc.sync.dma_start(brow[a][:], d_b[a].ap())
            cos_t = cpool.tile([P, SO, HD], BF16, name="cos_t")
            sin_t = cpool.tile([P, SO, HD], BF16, name="sin_t")
            urow_t = cpool.tile([1, H], F32R, name="urow_t")
            crow_t = cpool.tile([1, H], F32, name="crow_t")

            ones = cpool.tile([P, 1], F32, name="ones")
            nc.vector.memset(ones[:], 1.0)
            # [P, 2] so fp32r matmul free-dim-even ISA restrictions hold
            ones_r = cpool.tile([P, 2], F32R, name="ones_r")
            nc.vector.tensor_copy(ones_r[:], ones[:].to_broadcast((P, 2)))
            ones1pf = cpool.tile([1, P], F32, name="ones1pf")
            nc.vector.memset(ones1pf[:], 1.0)
            ones1p = cpool.tile([1, P], BF16, name="ones1p")
            nc.vector.tensor_copy(ones1p[:], ones1pf[:])
            eps_t = cpool.tile([P, 1], F32, name="eps_t")
            nc.vector.memset(eps_t[:], LN_EPS)

            c_b = cpool.tile([P, H], F32, name="c_b")

            # zero-padded q/k head tiles (double-buffered by hc parity);
            # rows HD:P stay zero forever -> scores contraction K=128
            pads = {}
            for nm in ("qA", "qB", "kA", "kB"):
                for par in range(2):
                    t = cpool.tile([P, S], BF16, name=f"pad_{nm}{par}")
                    pads[(nm, par)] = t

            # persistent big tensors
            tt = cpool.tile([P, SO, S], BF16, name="tt")
            q_nat = cpool.tile([P, SO, H], BF16, name="q_nat")
            k_nat = cpool.tile([P, SO, H], BF16, name="k_nat")
            v_ext = cpool.tile([P, SO, NH, HD + 1], BF16, name="v_ext")
            attn_T = cpool.tile([P, HO, S], BF16, name="attn_T")
            acc = cpool.tile([P, S], F32R, name="acc")
            acc2 = cpool.tile([P, S], F32R, name="acc2")

            def _rope_chunk(a_nat, so):
                ch = a_nat[:, so, :]
                ch3 = ch.rearrange("p (nh d) -> p nh d", d=HD)
                ch4 = ch.rearrange("p (nh hf dd) -> p nh hf dd", hf=2, dd=HD // 2)
                rot = s4.tile([P, H], BF16, tag="rope", bufs=3)
                rot4 = rot[:].rearrange("p (nh hf dd) -> p nh hf dd",
                                        hf=2, dd=HD // 2)
                rot3 = rot[:].rearrange("p (nh d) -> p nh d", d=HD)
                nc.vector.tensor_scalar_mul(rot4[:, :, 0, :], ch4[:, :, 1, :], -1.0)
                nc.vector.tensor_copy(rot4[:, :, 1, :], ch4[:, :, 0, :])
                cb = cos_t[:, so, :][:, None, :].to_broadcast((P, NH, HD))
                sb = sin_t[:, so, :][:, None, :].to_broadcast((P, NH, HD))
                nc.vector.tensor_tensor(ch3[:], ch3[:], cb, mybir.AluOpType.mult)
                nc.vector.tensor_tensor(rot3[:], rot3[:], sb, mybir.AluOpType.mult)
                nc.vector.tensor_tensor(ch[:], ch[:], rot[:], mybir.AluOpType.add)

            def project(a, a_nat=None, do_rope=False, defer=None):
                """a_nat [P, SO, H] (bf16) = X @ W_a + b_a, optional fused RoPE.

                Weights stream per-ko chunk; bias rides the accumulation as a
                K=1 rank-1 matmul; psum evictions alternate DVE/Act."""
                w_t = big.tile([P, HO, H], BF16, tag="w")
                for ko in range(HO):
                    nc.sync.dma_start(w_t[:, ko, :], d_w[a].ap()[:, ko, :])
                if a_nat is None:  # alloc after w_t so ring slots don't stall
                    a_nat = big.tile([P, SO, H], BF16, tag="w")
                xts = [None] * SO

                def load_xt(so):
                    t = s4.tile([P, HO, P], BF16, tag="xt", bufs=3)
                    nc.sync.dma_start(t[:], d_xt[a].ap()[:, so, :])
                    return t

                xts[0] = load_xt(0)
                if defer is not None:
                    defer()
                for so in range(SO):
                    if so + 1 < SO:
                        xts[so + 1] = load_xt(so + 1)
                    for nh in range(2):
                        ps = mmps.tile([P, 512], F32, tag="mm")
                        for ko in range(HO):
                            nc.tensor.matmul(
                                ps[:], xts[so][:, ko, :],
                                w_t[:, ko, nh * 512:(nh + 1) * 512],
                                start=(ko == 0), stop=False)
                        nc.tensor.matmul(
                            ps[:], ones1p[:],
                            brow[a][:, nh * 512:(nh + 1) * 512],
                            start=False, stop=True)
                        dst = a_nat[:, so, nh * 512:(nh + 1) * 512]
                        if nh == 0:
                            nc.vector.tensor_copy(dst, ps[:])
                        else:
                            nc.scalar.copy(dst, ps[:])
                    if do_rope:
                        _rope_chunk(a_nat, so)
                return a_nat

            def _band_sos(o0, o1):
                """so chunks whose s-range intersects [o0-BAND, o1+BAND)."""
                return [so for so in range(SO)
                        if so * P + P > o0 - BAND and so * P < o1 + BAND]

            def t_agg_v(v_nat):
                """V_ext [P, SO, NH, HD+1] (bf16) = T @ V with ones column."""
                nc.vector.tensor_copy(
                    v_ext[:, :, :, HD:HD + 1],
                    ones[:, None, None, :].to_broadcast((P, SO, NH, 1)))
                for sc in range(SO):
                    sos = _band_sos(sc * P, (sc + 1) * P)
                    for dh in range(2):
                        ps = mmps.tile([P, 512], F32, tag="mm")
                        for so in sos:
                            nc.tensor.matmul(
                                ps[:], tt[:, so, sc * P:(sc + 1) * P],
                                v_nat[:, so, dh * 512:(dh + 1) * 512],
                                start=(so == sos[0]), stop=(so == sos[-1]))
                        pvw = ps[:].rearrange("p (nh d) -> p nh d", d=HD)
                        nc.scalar.copy(
                            v_ext[:, sc, dh * 8:(dh + 1) * 8, 0:HD], pvw)

            # ---- prelude: projections + RoPE + V aggregation.
            # Deferred DMAs ride behind the V weight/x chunks so the PE
            # starts as early as possible.
            v_nat = project("v", defer=lambda: (
                [nc.sync.dma_start(tt[:, so, :], d_tt.ap()[:, so, :])
                 for so in range(SO)],
                nc.sync.dma_start(cos_t[:], d_cos.ap()),
                nc.sync.dma_start(sin_t[:], d_sin.ap()),
            ))
            t_agg_v(v_nat)
            project("q", q_nat, do_rope=True, defer=lambda: (
                [nc.sync.dma_start(pads[k][HD:P, :], d_zp.ap())
                 for k in pads],
                nc.sync.dma_start(urow_t[:], d_urow.ap()),
                nc.sync.dma_start(crow_t[:], d_crow.ap()),
                nc.gpsimd.partition_broadcast(c_b[:], crow_t[:]),
            ))
            project("k", k_nat, do_rope=True)

            # prefetch out-projection weights (gamma pre-folded on host)
            wg_t = big.tile([P, HO, H], BF16, tag="w")
            for ko in range(HO):
                nc.sync.dma_start(wg_t[:, ko, :], d_w["g"].ap()[:, ko, :])

            # ---- head-chunk loop: t_agg q/k -> scores -> exp -> PV -> stats
            def t_agg_qk(a_nat, padA, padB):
                for sh in range(2):
                    sos = _band_sos(sh * 512, (sh + 1) * 512)
                    ps = mmps.tile([P, 512], F32, tag="mm")
                    for so in sos:
                        nc.tensor.matmul(
                            ps[:], a_nat[:, so, hc * P:(hc + 1) * P],
                            tt[:, so, sh * 512:(sh + 1) * 512],
                            start=(so == sos[0]), stop=(so == sos[-1]))
                    qs = slice(sh * 512, (sh + 1) * 512)
                    nc.vector.tensor_copy(padA[0:HD, qs], ps[0:HD, :])
                    nc.vector.tensor_copy(padB[0:HD, qs], ps[HD:P, :])

            for hc in range(HO):
                par = hc & 1
                qpA, qpB = pads[("qA", par)], pads[("qB", par)]
                kpA, kpB = pads[("kA", par)], pads[("kB", par)]
                t_agg_qk(q_nat, qpA, qpB)
                t_agg_qk(k_nat, kpA, kpB)
                rb_c = s4.tile([P, S], F32, tag="s4")
                for hh in range(2):
                    h = 2 * hc + hh
                    off = hh * HD
                    qp = qpA if hh == 0 else qpB
                    kp = kpA if hh == 0 else kpB
                    for q2 in range(2):
                        pv = pvps.tile([P, 512], F32, tag="pv")
                        ets = []
                        for pr in range(5):
                            if pr < 4:
                                sc = scps.tile([P, 2, 512], F32, tag="sc")
                                for j in range(2):
                                    nc.tensor.matmul(
                                        sc[:, j, :],
                                        kp[:, (2 * pr + j) * P:(2 * pr + j + 1) * P],
                                        qp[:, q2 * 512:(q2 + 1) * 512],
                                        start=True, stop=True,
                                        skip_group_check=True)
                                e_t = s2.tile([P, 2, 512], BF16, tag="s2")
                                nc.scalar.activation(
                                    e_t[:], sc[:],
                                    mybir.ActivationFunctionType.Exp, scale=0.125)
                                ets.append(e_t)
                            if pr > 0:
                                for j in range(2):
                                    kc = 2 * (pr - 1) + j
                                    nc.tensor.matmul(
                                        pv[0:HD + 1, :], v_ext[:, kc, h, :],
                                        ets[pr - 1][:, j, :],
                                        start=(kc == 0), stop=(kc == SO - 1),
                                        skip_group_check=True)
                        # evict raw out + sums; broadcast sums along partitions
                        qs = slice(q2 * 512, (q2 + 1) * 512)
                        nc.vector.tensor_copy(attn_T[off:off + HD, hc, qs],
                                              pv[0:HD, :])
                        srow = s2.tile([1, 512], F32, tag="srow", bufs=2)
                        nc.vector.tensor_copy(srow[:], pv[HD:HD + 1, :])
                        if hh == 0:
                            nc.gpsimd.partition_broadcast(rb_c[0:HD, qs], srow[:])
                        else:
                            tmp = s2.tile([HD, 512], F32, tag="btmp", bufs=2)
                            nc.gpsimd.partition_broadcast(tmp[:], srow[:])
                            nc.vector.tensor_copy(rb_c[off:off + HD, qs], tmp[:])
                # chunk hc complete: normalize + accumulate LN stats
                rcp_c = s4.tile([P, S], F32, tag="s4")
                nc.vector.reciprocal_approx_fast(rcp_c[:], rb_c[:])
                nc.vector.tensor_tensor(attn_T[:, hc, :], attn_T[:, hc, :],
                                        rcp_c[:], mybir.AluOpType.mult)
                if hc == 0:
                    nc.vector.tensor_copy(acc[:], attn_T[:, 0, :])
                    nc.vector.tensor_tensor(acc2[:], attn_T[:, 0, :],
                                            attn_T[:, 0, :],
                                            mybir.AluOpType.mult)
                else:
                    nc.vector.tensor_tensor(acc[:], acc[:], attn_T[:, hc, :],
                                            mybir.AluOpType.add)
                    sqc = s4.tile([P, S], F32, tag="s4")
                    nc.vector.tensor_tensor(sqc[:], attn_T[:, hc, :],
                                            attn_T[:, hc, :],
                                            mybir.AluOpType.mult)
                    nc.vector.tensor_tensor(acc2[:], acc2[:], sqc[:],
                                            mybir.AluOpType.add)

            # ---- LayerNorm stats (no PE stall: all tiny matmuls)
            # s-partition-layout sums via ones-column matmuls (N=2 dup cols
            # to satisfy fp32r free-dim-even ISA restrictions)
            st_ps = mmps.tile([P, 32], F32, tag="mm")
            for so in range(SO):
                nc.tensor.matmul(st_ps[:, 4 * so:4 * so + 2],
                                 acc[:, so * P:(so + 1) * P], ones_r[:],
                                 start=True, stop=True, skip_group_check=True)
                nc.tensor.matmul(st_ps[:, 4 * so + 2:4 * so + 4],
                                 acc2[:, so * P:(so + 1) * P], ones_r[:],
                                 start=True, stop=True, skip_group_check=True)
            # row-layout sums for the rank-1 mu correction
            mu_row = cpool.tile([1, S], F32R, name="mu_row")
            for half in range(2):
                ps = mmps.tile([P, 512], F32, tag="mm")
                nc.tensor.matmul(ps[0:2, :], ones_r[:],
                                 acc[:, half * 512:(half + 1) * 512],
                                 start=True, stop=True, skip_group_check=True)
                nc.vector.tensor_copy(mu_row[:, half * 512:(half + 1) * 512],
                                      ps[0:1, :])
            # rstd[p, so] = 1/sqrt(E[z^2] - E[z]^2 + eps)
            st4 = st_ps[:].rearrange("p (so four) -> p so four", four=4)
            mu_s = s2.tile([P, 8], F32, tag="stat")
            nc.vector.tensor_scalar_mul(mu_s[:], st4[:, :, 0], 1.0 / H)
            ms_s = s2.tile([P, 8], F32, tag="stat")
            nc.vector.tensor_scalar_mul(ms_s[:], st4[:, :, 2], 1.0 / H)
            mu2 = s2.tile([P, 8], F32, tag="stat")
            nc.vector.tensor_tensor(mu2[:], mu_s[:], mu_s[:],
                                    mybir.AluOpType.mult)
            nc.vector.tensor_tensor(ms_s[:], ms_s[:], mu2[:],
                                    mybir.AluOpType.subtract)
            nc.scalar.activation(ms_s[:], ms_s[:],
                                 mybir.ActivationFunctionType.Sqrt,
                                 bias=eps_t[:])
            rstd = s2.tile([P, 8], F32, tag="stat")
            nc.vector.reciprocal_approx_fast(rstd[:], ms_s[:])

            # ---- out-projection: y = rstd * (z@Wg - mu (x) u) + c
            for so in range(SO):
                for nh in range(2):
                    ps = mmps.tile([P, 512], F32, tag="mm")
                    for ko in range(HO):
                        nc.tensor.matmul(
                            ps[:], attn_T[:, ko, so * P:(so + 1) * P],
                            wg_t[:, ko, nh * 512:(nh + 1) * 512],
                            start=(ko == 0), stop=False)
                    nc.tensor.matmul(
                        ps[:], mu_row[:, so * P:(so + 1) * P],
                        urow_t[:, nh * 512:(nh + 1) * 512],
                        start=False, stop=True)
                    ych = s2.tile([P, 512], F32, tag="s2")
                    nc.vector.tensor_scalar_mul(ych[:], ps[:],
                                                rstd[:, so:so + 1])
                    nc.vector.tensor_tensor(ych[:], ych[:],
                                            c_b[:, nh * 512:(nh + 1) * 512],
                                            mybir.AluOpType.add)
                    nc.sync.dma_start(
                        d_y.ap()[:, so, nh * 512:(nh + 1) * 512], ych[:])

    nc.compile()
    return nc


_NC = None


def _get_nc():
    global _NC
    if _NC is None:
        _NC = _build_program()
    return _NC


def _host_inputs(query, key, value, Wq, bq, Wk, bk, Wv, bv, Wo, bo,
                 temporal_weights, ln_gamma, ln_beta):
    T = _temporal_matrix(temporal_weights)
    tt_host = np.ascontiguousarray(  # TT[p, so, s'] = T[s', so*P+p]
        T.T.reshape(SO, P, S).transpose(1, 0, 2))
    cos, sin = _rope_tables()
    Wo64 = np.asarray(Wo, np.float64)
    gam = np.asarray(ln_gamma, np.float64)
    bet = np.asarray(ln_beta, np.float64)
    Wg = (gam[:, None] * Wo64).astype(np.float32)
    urow = -(Wg.astype(np.float64).sum(axis=0) / H).astype(np.float32)
    crow = (bet @ Wo64 + np.asarray(bo, np.float64)).astype(np.float32)
    common = {
        "w_v": _nat(np.asarray(Wv, np.float32)).astype(BF),
        "w_q": _nat(np.asarray(Wq, np.float32)).astype(BF),
        "w_k": _nat(np.asarray(Wk, np.float32)).astype(BF),
        "w_g": _nat(Wg).astype(BF),
        "b_v": np.asarray(bv, np.float32).reshape(1, H).astype(BF),
        "b_q": np.asarray(bq, np.float32).reshape(1, H).astype(BF),
        "b_k": np.asarray(bk, np.float32).reshape(1, H).astype(BF),
        "tt": tt_host.astype(BF),
        "zpad": np.zeros((HD, S), BF),
        "cos_t": _nat(cos).astype(BF),
        "sin_t": _nat(sin).astype(BF),
        "urow": urow.reshape(1, H),
        "crow": crow.reshape(1, H),
    }
    in_maps = []
    for c in range(N_CORES):
        m = dict(common)
        m["xt_q"] = _xt_chunks(np.asarray(query[c], np.float32)).astype(BF)
        m["xt_k"] = _xt_chunks(np.asarray(key[c], np.float32)).astype(BF)
        m["xt_v"] = _xt_chunks(np.asarray(value[c], np.float32)).astype(BF)
        in_maps.append(m)
    return in_maps


def kernel(query, key, value, Wq, bq, Wk, bk, Wv, bv, Wo, bo,
           temporal_weights, ln_gamma, ln_beta):
    in_maps = _host_inputs(query, key, value, Wq, bq, Wk, bk, Wv, bv, Wo, bo,
                           temporal_weights, ln_gamma, ln_beta)
    nc = _get_nc()
    res = run_bass_kernel_spmd(nc, in_maps, list(range(N_CORES)))
    out = np.empty((B, S, H), np.float32)
    for c in range(N_CORES):
        y = res.results[c]["y"]  # [P, SO, H]
        out[c] = y.transpose(1, 0, 2).reshape(S, H)
    return out


# revision 27
# speedup vs baseline: 1.5113x; 1.0700x over previous
"""HSTU-style attention block (RoPE + multi-scale temporal agg + SDPA + LN + out-proj)
for Trainium2, data-parallel over batch across 8 NeuronCores.

Per-core layout strategy (batch element per core), v2:
  - all SBUF operands bf16 (same PE col-rate as f32r at N>=512, half the DMA
    and SBUF footprint); PSUM accumulation stays f32
  - projections: host pre-transposed X as lhsT, per-ko weight-chunk DMAs so the
    PE starts ~1.5us in; bias folded into the matmul as a K=1 rank-1 accumulate
  - temporal aggregation (T @ .) for Q/K runs per head-chunk INSIDE the head
    loop, evicted straight into persistent zero-padded SBUF tiles (no DRAM
    spill roundtrip); V aggregated in the prelude with a ones column so softmax
    denominators ride the PV matmul
  - attention: scores^T per head with contraction zero-padded to K=128 (keeps
    the PE activity monitor / clock boost engaged), Exp on the scalar engine in
    2-PSUM-bank pairs (halves Act instruction overhead), PV accumulation over
    key chunks; softmax normalization + LayerNorm stats fold into the head loop
  - LayerNorm folded into the out-projection: host computes Wg = gamma*Wo,
    u = colsum(Wg), c = beta@Wo + bo; device accumulates z@Wg - mu (x) u in
    PSUM (rank-1 correction), then scales by rstd (per-partition scalar) and
    adds c. The PE never waits on LayerNorm.
"""

import numpy as np
import ml_dtypes
import concourse.mybir as mybir
import concourse.tile as tile
from concourse import bacc
from concourse.bass_utils import run_bass_kernel_spmd

B, S, H, NH = 8, 1024, 1024, 16
HD = H // NH  # 64
P = 128
SO = S // P  # 8
HO = H // P  # 8
N_SCALES = 4
LN_EPS = 1e-5
F32 = mybir.dt.float32
F32R = mybir.dt.float32r
BF16 = mybir.dt.bfloat16
BF = ml_dtypes.bfloat16

N_CORES = 8
BAND = 12  # T[s', s] == 0 for |s' - s| > 11 (structural)


# ---------------------------------------------------------------- host helpers
def _softmax_np(x):
    x = np.asarray(x, np.float64)
    e = np.exp(x - x.max())
    return e / e.sum()


def _temporal_matrix(temporal_weights):
    """[S, S] matrix T with (T @ x) == temporal_agg(x) along the sequence axis."""
    w = _softmax_np(temporal_weights)
    T = np.eye(S, dtype=np.float64) * w[0]
    for scale in range(1, N_SCALES):
        p = max(1, S // (2 ** scale))
        k = S // p
        pool = np.zeros((p, S), dtype=np.float64)
        for j in range(p):
            pool[j, j * k:(j + 1) * k] = 1.0 / k
        coord = (np.arange(S, dtype=np.float64) + 0.5) * (p / S) - 0.5
        coord = np.clip(coord, 0.0, None)
        i0 = np.minimum(np.floor(coord).astype(np.int64), p - 1)
        i1 = np.minimum(i0 + 1, p - 1)
        lam = (coord - i0).astype(np.float32).astype(np.float64)
        interp = np.zeros((S, p), dtype=np.float64)
        interp[np.arange(S), i0] += 1.0 - lam
        interp[np.arange(S), i1] += lam
        T += w[scale] * (interp @ pool)
    return T.astype(np.float32)


def _rope_tables():
    inv_freq = 1.0 / (10000.0 ** (np.arange(0, HD, 2, dtype=np.float64) / HD))
    freqs = np.arange(S, dtype=np.float64)[:, None] * inv_freq[None, :]
    cos = np.repeat(np.cos(freqs), 2, axis=-1).astype(np.float32)  # [S, HD]
    sin = np.repeat(np.sin(freqs), 2, axis=-1).astype(np.float32)
    return cos, sin


def _nat(x):
    """[S, D] -> [P, S//P, D] with x[so*P+p, d] = out[p, so, d]."""
    return np.ascontiguousarray(x.reshape(SO, P, x.shape[-1]).transpose(1, 0, 2))


def _xt_chunks(x):
    """[S, H] -> [P, SO, HO*P] with out[p, so, ho*P + i] = x[so*P + i, ho*P + p]."""
    return np.ascontiguousarray(
        x.reshape(SO, P, HO, P).transpose(3, 0, 2, 1).reshape(P, SO, H))


# ---------------------------------------------------------------- bass program
def _build_program(zero_bias=False, zero_c=False):
    """zero_bias/zero_c: specialized variants that skip structurally-zero
    bias and output-constant work (selected at runtime from actual inputs;
    the general variant handles arbitrary values)."""
    nc = bacc.Bacc("TRN2", target_bir_lowering=False, debug=False)

    d_xt = {a: nc.dram_tensor(f"xt_{a}", [P, SO, H], BF16, kind="ExternalInput")
            for a in ("v", "q", "k")}
    d_w = {a: nc.dram_tensor(f"w_{a}", [P, HO, H], BF16, kind="ExternalInput")
           for a in ("v", "q", "k", "g")}
    d_b = {a: nc.dram_tensor(f"b_{a}", [1, H], BF16, kind="ExternalInput")
           for a in ("v", "q", "k")}
    d_tt = nc.dram_tensor("tt", [P, SO, S], BF16, kind="ExternalInput")
    d_cos = nc.dram_tensor("cos_t", [P, SO, HD], BF16, kind="ExternalInput")
    d_sin = nc.dram_tensor("sin_t", [P, SO, HD], BF16, kind="ExternalInput")
    d_urow = nc.dram_tensor("urow", [1, H], F32R, kind="ExternalInput")
    d_crow = nc.dram_tensor("crow", [1, H], F32, kind="ExternalInput")
    d_zp = nc.dram_tensor("zpad", [HD, S], BF16, kind="ExternalInput")
    d_y = nc.dram_tensor("y", [P, SO, H], F32, kind="ExternalOutput")

    with tile.TileContext(nc) as tc:
        with (
            tc.tile_pool(name="const", bufs=1) as cpool,
            tc.tile_pool(name="big", bufs=2) as big,
            tc.tile_pool(name="s4", bufs=4) as s4,
            tc.tile_pool(name="s2", bufs=4) as s2,
            tc.tile_pool(name="mm_ps", bufs=2, space="PSUM") as mmps,
            tc.tile_pool(name="sc_ps", bufs=2, space="PSUM") as scps,
            tc.tile_pool(name="pv_ps", bufs=2, space="PSUM") as pvps,
        ):
            # ---- constants / persistent state
            brow = {a: cpool.tile([1, H], BF16, name=f"brow_{a}")
                    for a in ("v", "q", "k")}
            cos_t = cpool.tile([P, SO, HD], BF16, name="cos_t")
            sin_t = cpool.tile([P, SO, HD], BF16, name="sin_t")
            urow_t = cpool.tile([1, H], F32R, name="urow_t")
            crow_t = cpool.tile([1, H], F32, name="crow_t")

            ones = cpool.tile([P, 1], F32, name="ones")
            nc.vector.memset(ones[:], 1.0)
            # [P, 2] so fp32r matmul free-dim-even ISA restrictions hold
            ones_r = cpool.tile([P, 2], F32R, name="ones_r")
            nc.vector.tensor_copy(ones_r[:], ones[:].to_broadcast((P, 2)))
            ones1pf = cpool.tile([1, P], F32, name="ones1pf")
            nc.vector.memset(ones1pf[:], 1.0)
            ones1p = cpool.tile([1, P], BF16, name="ones1p")
            nc.vector.tensor_copy(ones1p[:], ones1pf[:])
            eps_t = cpool.tile([P, 1], F32, name="eps_t")
            nc.vector.memset(eps_t[:], LN_EPS)

            c_b = cpool.tile([P, H], F32, name="c_b")

            # zero-padded q/k head tiles (double-buffered by hc parity);
            # rows HD:P stay zero forever -> scores contraction K=128
            pads = {}
            for nm in ("qA", "qB", "kA", "kB"):
                for par in range(2):
                    t = cpool.tile([P, S], BF16, name=f"pad_{nm}{par}")
                    pads[(nm, par)] = t

            # persistent big tensors
            tt = cpool.tile([P, SO, S], BF16, name="tt")
            q_nat = cpool.tile([P, SO, H], BF16, name="q_nat")
            k_nat = cpool.tile([P, SO, H], BF16, name="k_nat")
            v_ext = cpool.tile([P, SO, NH, HD + 1], BF16, name="v_ext")
            attn_T = cpool.tile([P, HO, S], BF16, name="attn_T")
            acc = cpool.tile([P, S], F32R, name="acc")
            acc2 = cpool.tile([P, S], F32R, name="acc2")

            def _rope_chunk(a_nat, so):
                ch = a_nat[:, so, :]
                ch3 = ch.rearrange("p (nh d) -> p nh d", d=HD)
                ch4 = ch.rearrange("p (nh hf dd) -> p nh hf dd", hf=2, dd=HD // 2)
                rot = s4.tile([P, H], BF16, tag="rope", bufs=3)
                rot4 = rot[:].rearrange("p (nh hf dd) -> p nh hf dd",
                                        hf=2, dd=HD // 2)
                rot3 = rot[:].rearrange("p (nh d) -> p nh d", d=HD)
                nc.vector.tensor_scalar_mul(rot4[:, :, 0, :], ch4[:, :, 1, :], -1.0)
                nc.vector.tensor_copy(rot4[:, :, 1, :], ch4[:, :, 0, :])
                cb = cos_t[:, so, :][:, None, :].to_broadcast((P, NH, HD))
                sb = sin_t[:, so, :][:, None, :].to_broadcast((P, NH, HD))
                nc.vector.tensor_tensor(ch3[:], ch3[:], cb, mybir.AluOpType.mult)
                nc.vector.tensor_tensor(rot3[:], rot3[:], sb, mybir.AluOpType.mult)
                nc.vector.tensor_tensor(ch[:], ch[:], rot[:], mybir.AluOpType.add)

            def project(a, a_nat=None, do_rope=False, defer=None):
                """a_nat [P, SO, H] (bf16) = X @ W_a + b_a, optional fused RoPE.

                Weights stream per-ko chunk; bias rides the accumulation as a
                K=1 rank-1 matmul; psum evictions alternate DVE/Act."""
                w_t = big.tile([P, HO, H], BF16, tag="w")
                if a_nat is None:  # alloc after w_t so ring slots don't stall
                    a_nat = big.tile([P, SO, H], BF16, tag="w")
                xts = [None] * SO

                def load_xt(so):
                    t = s4.tile([P, HO, P], BF16, tag="xt", bufs=3)
                    nc.sync.dma_start(t[:], d_xt[a].ap()[:, so, :])
                    return t

                # DMA issue is serial (~0.6us each on the sync queue): x
                # chunk 0 and weight chunks go first so the PE starts ASAP
                xts[0] = load_xt(0)
                for ko in range(HO):
                    nc.sync.dma_start(w_t[:, ko, :], d_w[a].ap()[:, ko, :])
                if not zero_bias:
                    nc.sync.dma_start(brow[a][:], d_b[a].ap())
                if defer is not None:
                    defer()
                for so in range(SO):
                    if so + 1 < SO:
                        xts[so + 1] = load_xt(so + 1)
                    for nh in range(2):
                        ps = mmps.tile([P, 512], F32, tag="mm")
                        for ko in range(HO):
                            nc.tensor.matmul(
                                ps[:], xts[so][:, ko, :],
                                w_t[:, ko, nh * 512:(nh + 1) * 512],
                                start=(ko == 0),
                                stop=(zero_bias and ko == HO - 1))
                        if not zero_bias:
                            nc.tensor.matmul(
                                ps[:], ones1p[:],
                                brow[a][:, nh * 512:(nh + 1) * 512],
                                start=False, stop=True)
                        dst = a_nat[:, so, nh * 512:(nh + 1) * 512]
                        if nh == 0:
                            nc.vector.tensor_copy(dst, ps[:])
                        else:
                            nc.scalar.copy(dst, ps[:])
                    if do_rope:
                        _rope_chunk(a_nat, so)
                return a_nat

            def _band_sos(o0, o1):
                """so chunks whose s-range intersects [o0-BAND, o1+BAND)."""
                return [so for so in range(SO)
                        if so * P + P > o0 - BAND and so * P < o1 + BAND]

            def t_agg_v(v_nat):
                """V_ext [P, SO, NH, HD+1] (bf16) = T @ V with ones column."""
                nc.vector.tensor_copy(
                    v_ext[:, :, :, HD:HD + 1],
                    ones[:, None, None, :].to_broadcast((P, SO, NH, 1)))
                for sc in range(SO):
                    sos = _band_sos(sc * P, (sc + 1) * P)
                    for dh in range(2):
                        ps = mmps.tile([P, 512], F32, tag="mm")
                        for so in sos:
                            nc.tensor.matmul(
                                ps[:], tt[:, so, sc * P:(sc + 1) * P],
                                v_nat[:, so, dh * 512:(dh + 1) * 512],
                                start=(so == sos[0]), stop=(so == sos[-1]))
                        pvw = ps[:].rearrange("p (nh d) -> p nh d", d=HD)
                        nc.scalar.copy(
                            v_ext[:, sc, dh * 8:(dh + 1) * 8, 0:HD], pvw)

            # ---- prelude: projections + RoPE + V aggregation.
            # Deferred DMAs ride behind the V weight/x chunks so the PE
            # starts as early as possible.
            v_nat = project("v", defer=lambda: (
                [nc.sync.dma_start(tt[:, so, :], d_tt.ap()[:, so, :])
                 for so in range(SO)],
                nc.sync.dma_start(cos_t[:], d_cos.ap()),
                nc.sync.dma_start(sin_t[:], d_sin.ap()),
            ))
            t_agg_v(v_nat)
            def _defer_q():
                for k in pads:
                    nc.sync.dma_start(pads[k][HD:P, :], d_zp.ap())
                nc.sync.dma_start(urow_t[:], d_urow.ap())
                if not zero_c:
                    nc.sync.dma_start(crow_t[:], d_crow.ap())
                    nc.gpsimd.partition_broadcast(c_b[:], crow_t[:])

            project("q", q_nat, do_rope=True, defer=_defer_q)
            project("k", k_nat, do_rope=True)

            # prefetch out-projection weights (gamma pre-folded on host)
            wg_t = big.tile([P, HO, H], BF16, tag="w")
            for ko in range(HO):
                nc.sync.dma_start(wg_t[:, ko, :], d_w["g"].ap()[:, ko, :])

            # ---- head-chunk loop: t_agg q/k -> scores -> exp -> PV -> stats
            def t_agg_qk(a_nat, padA, padB):
                # 256-col output blocks: the +-BAND structure of T needs only
                # 3-4 so-chunks per block (vs 5 at 512) -> fewer PE cols
                for sh in range(2):
                    ps = mmps.tile([P, 512], F32, tag="mm")
                    for half in range(2):
                        o0 = (2 * sh + half) * 256
                        sos = _band_sos(o0, o0 + 256)
                        for so in sos:
                            nc.tensor.matmul(
                                ps[:, half * 256:(half + 1) * 256],
                                a_nat[:, so, hc * P:(hc + 1) * P],
                                tt[:, so, o0:o0 + 256],
                                start=(so == sos[0]), stop=(so == sos[-1]),
                                skip_group_check=True)
                    qs = slice(sh * 512, (sh + 1) * 512)
                    nc.vector.tensor_copy(padA[0:HD, qs], ps[0:HD, :])
                    nc.vector.tensor_copy(padB[0:HD, qs], ps[HD:P, :])

            for hc in range(HO):
                par = hc & 1
                qpA, qpB = pads[("qA", par)], pads[("qB", par)]
                kpA, kpB = pads[("kA", par)], pads[("kB", par)]
                t_agg_qk(q_nat, qpA, qpB)
                t_agg_qk(k_nat, kpA, kpB)
                rb_c = s4.tile([P, S], F32, tag="s4")
                for hh in range(2):
                    h = 2 * hc + hh
                    off = hh * HD
                    qp = qpA if hh == 0 else qpB
                    kp = kpA if hh == 0 else kpB
                    for q2 in range(2):
                        pv = pvps.tile([P, 512], F32, tag="pv")
                        ets = []
                        for pr in range(5):
                            if pr < 4:
                                sc = scps.tile([P, 2, 512], F32, tag="sc")
                                for j in range(2):
                                    nc.tensor.matmul(
                                        sc[:, j, :],
                                        kp[:, (2 * pr + j) * P:(2 * pr + j + 1) * P],
                                        qp[:, q2 * 512:(q2 + 1) * 512],
                                        start=True, stop=True,
                                        skip_group_check=True)
                                e_t = s2.tile([P, 2, 512], BF16, tag="s2")
                                nc.scalar.activation(
                                    e_t[:], sc[:],
                                    mybir.ActivationFunctionType.Exp, scale=0.125)
                                ets.append(e_t)
                            if pr > 0:
                                for j in range(2):
                                    kc = 2 * (pr - 1) + j
                                    nc.tensor.matmul(
                                        pv[0:HD + 1, :], v_ext[:, kc, h, :],
                                        ets[pr - 1][:, j, :],
                                        start=(kc == 0), stop=(kc == SO - 1),
                                        skip_group_check=True)
                        # evict raw out + sums; broadcast sums along partitions
                        qs = slice(q2 * 512, (q2 + 1) * 512)
                        nc.vector.tensor_copy(attn_T[off:off + HD, hc, qs],
                                              pv[0:HD, :])
                        srow = s2.tile([1, 512], F32, tag="srow", bufs=2)
                        nc.vector.tensor_copy(srow[:], pv[HD:HD + 1, :])
                        if hh == 0:
                            nc.gpsimd.partition_broadcast(rb_c[0:HD, qs], srow[:])
                        else:
                            tmp = s2.tile([HD, 512], F32, tag="btmp", bufs=2)
                            nc.gpsimd.partition_broadcast(tmp[:], srow[:])
                            nc.vector.tensor_copy(rb_c[off:off + HD, qs], tmp[:])
                # chunk hc complete: normalize + accumulate LN stats.
                # hc=7 skips the acc2 += sq step (sq7 feeds the stats matmul
                # directly) to shorten the DVE chain gating the tail.
                rcp_c = s4.tile([P, S], F32, tag="s4")
                nc.vector.reciprocal_approx_fast(rcp_c[:], rb_c[:])
                nc.vector.tensor_tensor(attn_T[:, hc, :], attn_T[:, hc, :],
                                        rcp_c[:], mybir.AluOpType.mult)
                if hc == 0:
                    nc.vector.tensor_copy(acc[:], attn_T[:, 0, :])
                    nc.vector.tensor_tensor(acc2[:], attn_T[:, 0, :],
                                            attn_T[:, 0, :],
                                            mybir.AluOpType.mult)
                else:
                    nc.vector.tensor_tensor(acc[:], acc[:], attn_T[:, hc, :],
                                            mybir.AluOpType.add)
                    sqc = s4.tile([P, S], F32R, tag="s4")
                    nc.vector.tensor_tensor(sqc[:], attn_T[:, hc, :],
                                            attn_T[:, hc, :],
                                            mybir.AluOpType.mult)
                    if hc < HO - 1:
                        nc.vector.tensor_tensor(acc2[:], acc2[:], sqc[:],
                                                mybir.AluOpType.add)
                    else:
                        sq7 = sqc

            # ---- LayerNorm stats (tiny matmuls; psums from the pv pool so
            # the out-projection's mm ring can't WAR-block on the rstd chain)
            # row-layout sums for the rank-1 mu correction (needs acc only)
            mu_row = cpool.tile([1, S], F32R, name="mu_row")
            for half in range(2):
                ps = pvps.tile([P, 512], F32, tag="pv")
                nc.tensor.matmul(ps[0:2, :], ones_r[:],
                                 acc[:, half * 512:(half + 1) * 512],
                                 start=True, stop=True, skip_group_check=True)
                nc.vector.tensor_copy(mu_row[:, half * 512:(half + 1) * 512],
                                      ps[0:1, :])
            # s-partition-layout sums via ones-column matmuls (N=2 dup cols
            # to satisfy fp32r free-dim-even ISA restrictions)
            st_ps = pvps.tile([P, 32], F32, tag="pv")
            for so in range(SO):
                nc.tensor.matmul(st_ps[:, 4 * so:4 * so + 2],
                                 acc[:, so * P:(so + 1) * P], ones_r[:],
                                 start=True, stop=True, skip_group_check=True)
                nc.tensor.matmul(st_ps[:, 4 * so + 2:4 * so + 4],
                                 acc2[:, so * P:(so + 1) * P], ones_r[:],
                                 start=True, stop=False, skip_group_check=True)
                nc.tensor.matmul(st_ps[:, 4 * so + 2:4 * so + 4],
                                 sq7[:, so * P:(so + 1) * P], ones_r[:],
                                 start=False, stop=True, skip_group_check=True)
            # rstd[p, so] = 1/sqrt(E[z^2] - E[z]^2 + eps)
            st4 = st_ps[:].rearrange("p (so four) -> p so four", four=4)
            mu_s = s2.tile([P, 8], F32, tag="stat")
            nc.vector.tensor_scalar_mul(mu_s[:], st4[:, :, 0], 1.0 / H)
            ms_s = s2.tile([P, 8], F32, tag="stat")
            nc.vector.tensor_scalar_mul(ms_s[:], st4[:, :, 2], 1.0 / H)
            mu2 = s2.tile([P, 8], F32, tag="stat")
            nc.vector.tensor_tensor(mu2[:], mu_s[:], mu_s[:],
                                    mybir.AluOpType.mult)
            nc.vector.tensor_tensor(ms_s[:], ms_s[:], mu2[:],
                                    mybir.AluOpType.subtract)
            nc.scalar.activation(ms_s[:], ms_s[:],
                                 mybir.ActivationFunctionType.Sqrt,
                                 bias=eps_t[:])
            rstd = s2.tile([P, 8], F32, tag="stat")
            nc.vector.reciprocal_approx_fast(rstd[:], ms_s[:])

            # ---- out-projection: y = rstd * (z@Wg - mu (x) u) + c
            for so in range(SO):
                for nh in range(2):
                    ps = mmps.tile([P, 512], F32, tag="mm")
                    for ko in range(HO):
                        nc.tensor.matmul(
                            ps[:], attn_T[:, ko, so * P:(so + 1) * P],
                            wg_t[:, ko, nh * 512:(nh + 1) * 512],
                            start=(ko == 0), stop=False)
                    nc.tensor.matmul(
                        ps[:], mu_row[:, so * P:(so + 1) * P],
                        urow_t[:, nh * 512:(nh + 1) * 512],
                        start=False, stop=True)
                    ych = s2.tile([P, 512], F32, tag="s2")
                    nc.scalar.mul(ych[:], ps[:], rstd[:, so:so + 1])
                    if not zero_c:
                        nc.vector.tensor_tensor(
                            ych[:], ych[:],
                            c_b[:, nh * 512:(nh + 1) * 512],
                            mybir.AluOpType.add)
                    nc.sync.dma_start(
                        d_y.ap()[:, so, nh * 512:(nh + 1) * 512], ych[:])

    nc.compile()
    return nc


_NC = {}


def _get_nc(zero_bias=True, zero_c=True):
    key = (zero_bias, zero_c)
    if key not in _NC:
        _NC[key] = _build_program(zero_bias=zero_bias, zero_c=zero_c)
    return _NC[key]


def _host_inputs(query, key, value, Wq, bq, Wk, bk, Wv, bv, Wo, bo,
                 temporal_weights, ln_gamma, ln_beta):
    T = _temporal_matrix(temporal_weights)
    tt_host = np.ascontiguousarray(  # TT[p, so, s'] = T[s', so*P+p]
        T.T.reshape(SO, P, S).transpose(1, 0, 2))
    cos, sin = _rope_tables()
    Wo64 = np.asarray(Wo, np.float64)
    gam = np.asarray(ln_gamma, np.float64)
    bet = np.asarray(ln_beta, np.float64)
    Wg = (gam[:, None] * Wo64).astype(np.float32)
    urow = -(Wg.astype(np.float64).sum(axis=0) / H).astype(np.float32)
    crow = (bet @ Wo64 + np.asarray(bo, np.float64)).astype(np.float32)
    common = {
        "w_v": _nat(np.asarray(Wv, np.float32)).astype(BF),
        "w_q": _nat(np.asarray(Wq, np.float32)).astype(BF),
        "w_k": _nat(np.asarray(Wk, np.float32)).astype(BF),
        "w_g": _nat(Wg).astype(BF),
        "b_v": np.asarray(bv, np.float32).reshape(1, H).astype(BF),
        "b_q": np.asarray(bq, np.float32).reshape(1, H).astype(BF),
        "b_k": np.asarray(bk, np.float32).reshape(1, H).astype(BF),
        "tt": tt_host.astype(BF),
        "zpad": np.zeros((HD, S), BF),
        "cos_t": _nat(cos).astype(BF),
        "sin_t": _nat(sin).astype(BF),
        "urow": urow.reshape(1, H),
        "crow": crow.reshape(1, H),
    }
    in_maps = []
    for c in range(N_CORES):
        m = dict(common)
        m["xt_q"] = _xt_chunks(np.asarray(query[c], np.float32)).astype(BF)
        m["xt_k"] = _xt_chunks(np.asarray(key[c], np.float32)).astype(BF)
        m["xt_v"] = _xt_chunks(np.asarray(value[c], np.float32)).astype(BF)
        in_maps.append(m)
    return in_maps


def kernel(query, key, value, Wq, bq, Wk, bk, Wv, bv, Wo, bo,
           temporal_weights, ln_gamma, ln_beta):
    in_maps = _host_inputs(query, key, value, Wq, bq, Wk, bk, Wv, bv, Wo, bo,
                           temporal_weights, ln_gamma, ln_beta)
    zero_bias = not (np.any(np.asarray(bq)) or np.any(np.asarray(bk))
                     or np.any(np.asarray(bv)))
    zero_c = not np.any(in_maps[0]["crow"])
    nc = _get_nc(zero_bias=zero_bias, zero_c=zero_c)
    res = run_bass_kernel_spmd(nc, in_maps, list(range(N_CORES)))
    out = np.empty((B, S, H), np.float32)
    for c in range(N_CORES):
        y = res.results[c]["y"]  # [P, SO, H]
        out[c] = y.transpose(1, 0, 2).reshape(S, H)
    return out
